# revision 1
# baseline (speedup 1.0000x reference)
"""Trainium2 Bass kernel for a dense transformer block (LN -> QKV -> attention ->
out-proj -> LN -> FFN with exact GELU, no residuals).

Sharding: pure data parallelism — batch 8 across 8 NeuronCores, one batch element
per core. Each core runs the full block on its [1024, 1024] token slab.

On-chip dataflow (per core):
  - LN1 in token-major fp32 (bn_stats); normalized output cast to bf16 and
    transposed to feature-major xnT [D, tokens] on the PE (bf16 transpose-mode).
  - All GEMMs in bf16 with fp32 PSUM accumulation. LayerNorm gains fold into the
    weights on the host; LN biases fold into per-feature GEMM biases. The
    1/sqrt(dh) attention scale folds into the Q projection.
  - Q/K produced feature-major per head pair (streamed), V token-major with a
    ones-column per head so attention@V also yields the softmax denominator.
  - Scores are computed k-major (scoresT) so the exp output feeds attention@V
    directly with no transpose; softmax skips max-subtraction (|scores| < ~3).
  - Denominator reciprocal is partition-broadcast by GPSIMD (no DMA).
  - x2 token-major, LN2 like LN1, FFN1 feature-major with fused bias+GELU on
    ACT, FFN2 token-major.

DMA discipline (hardware caps each DMA descriptor at ~2 sem waits, and Tile
adds one flow-control wait per DMA once queues saturate, so every DMA must
carry at most ONE data wait):
  - Weights live in 4 permanent 16KB "wslot" tag slots (wq/wk/wv/wout ->
    w1 quarters -> w2 quarters). Between tenants the slot is fully memset by
    GPSIMD: engine instructions have no wait caps and become the slot's sole
    last-writer, so the next load carries a single WAW wait.
  - Activation pools recycle SBUF freely but are only ever written by compute
    engines; output staging is a permanent pool so stores see only DVE deps.
"""

import numpy as np
import ml_dtypes

B, N, D = 8, 1024, 1024
H, DH = 16, 64
MLP = 4096
EPS = 1e-5
P = 128
NCORES = 8
TT = N // P    # 8 token tiles
DC = D // P    # 8 d-chunks
MT = MLP // P  # 32 mlp tiles


def build_bass(gelu_mode="gelu"):
    import concourse.bass as bass
    import concourse.mybir as mybir
    import concourse.tile as tile
    from concourse import bacc
    from concourse.masks import make_identity

    f32 = mybir.dt.float32
    bf16 = mybir.dt.bfloat16
    AF = mybir.ActivationFunctionType
    OP = mybir.AluOpType

    nc = bacc.Bacc()

    x_d = nc.declare_dram_parameter("x", [N, D], bf16, isOutput=False)
    wq_d = nc.declare_dram_parameter("wq", [D, D], bf16, isOutput=False)
    wk_d = nc.declare_dram_parameter("wk", [D, D], bf16, isOutput=False)
    wv_d = nc.declare_dram_parameter("wv", [D, D], bf16, isOutput=False)
    wo_d = nc.declare_dram_parameter("wo", [D, D], bf16, isOutput=False)
    w1_d = nc.declare_dram_parameter("w1", [D, MLP], bf16, isOutput=False)
    w2_d = nc.declare_dram_parameter("w2", [MLP, D], bf16, isOutput=False)
    bq_d = nc.declare_dram_parameter("bq", [D], f32, isOutput=False)
    bk_d = nc.declare_dram_parameter("bk", [D], f32, isOutput=False)
    bv_d = nc.declare_dram_parameter("bv", [D], f32, isOutput=False)
    bo_d = nc.declare_dram_parameter("bo", [D], f32, isOutput=False)
    bh_d = nc.declare_dram_parameter("bh", [MLP], f32, isOutput=False)
    b2_d = nc.declare_dram_parameter("b2", [D], f32, isOutput=False)
    out_d = nc.declare_dram_parameter("out", [N, D], f32, isOutput=True)

    gelu_func = AF.Gelu if gelu_mode == "gelu" else AF.Identity

    with tile.TileContext(nc) as tc:
        # ---- permanent pools (left stack bottom) ----
        const = tc.alloc_tile_pool(name="const", bufs=1)
        stats = tc.alloc_tile_pool(name="stats", bufs=4)
        psum = tc.alloc_tile_pool(name="psum", bufs=8, space="PSUM")
        wslot = tc.alloc_tile_pool(name="wslot", bufs=4)   # 4 x 16KB weight slots
        outp = tc.alloc_tile_pool(name="outp", bufs=2)

        counter = [0]

        def uniq(prefix):
            counter[0] += 1
            return f"{prefix}{counter[0]}"

        def ps_tile():
            return psum.tile([P, 512], f32, tag="ps", name=uniq("ps"))

        def wtile(shape):
            return wslot.tile(shape, bf16, tag="w", name=uniq("w"))

        def qpad(n, after_ap=None):
            """Emit n dummy HWDGE DMAs (virgin tiny targets) to rotate the
            round-robin queue assignment, so the next real load lands on the
            same queue as its slot's previous tenant load (merging the
            slot-WAW wait with the prior-same-queue wait). after_ap pins the
            pads' schedule position via a RAW dependency."""
            for _ in range(n):
                if after_ap is None:
                    d = const.tile([1, 4], f32, tag=uniq("pad"),
                                   name=uniq("padt"))
                    nc.sync.dma_start(d, bq_d[None, 0:4])
                else:
                    d = const.tile([1, 4], bf16, tag=uniq("pad"),
                                   name=uniq("padt"))
                    nc.sync.dma_start(d, after_ap)

        eps_t = const.tile([P, 1], f32, tag="eps")
        nc.vector.memset(eps_t, EPS)
        bq_sb = const.tile([P, DC], f32, tag="bq")
        nc.sync.dma_start(bq_sb, bq_d[:].rearrange("(o p) -> p o", p=P))
        bk_sb = const.tile([P, DC], f32, tag="bk")
        nc.sync.dma_start(bk_sb, bk_d[:].rearrange("(o p) -> p o", p=P))
        bh_sb = const.tile([P, MT], f32, tag="bh")
        nc.sync.dma_start(bh_sb, bh_d[:].rearrange("(o p) -> p o", p=P))
        bv_b = const.tile([P, D], bf16, tag="bvb")
        nc.gpsimd.dma_start(bv_b, bv_d[None, :].to_broadcast([P, D]))
        bo_b = const.tile([P, D], bf16, tag="bob")
        nc.gpsimd.dma_start(bo_b, bo_d[None, :].to_broadcast([P, D]))
        b2_b = const.tile([P, D], bf16, tag="b2b")
        nc.gpsimd.dma_start(b2_b, b2_d[None, :].to_broadcast([P, D]))
        ident = const.tile([P, P], bf16, tag="ident")
        make_identity(nc, ident)
        ones64f = const.tile([1, 64], f32, tag="ones64f")
        nc.vector.memset(ones64f, 1.0)
        ones64 = const.tile([1, 64], mybir.dt.float32r, tag="ones64")
        nc.vector.tensor_copy(ones64, ones64f)

        # weight slots round 1: wq, wk, wv, wout (all virgin -> clean loads)
        wq_sb = wtile([P, DC, D])
        nc.sync.dma_start(wq_sb, wq_d[:, :].rearrange("(o p) f -> p o f", p=P))
        wk_sb = wtile([P, DC, D])
        nc.sync.dma_start(wk_sb, wk_d[:, :].rearrange("(o p) f -> p o f", p=P))
        wv_sb = wtile([P, DC, D])
        nc.sync.dma_start(wv_sb, wv_d[:, :].rearrange("(o p) f -> p o f", p=P))
        wo_sb = wtile([P, DC, D])
        nc.sync.dma_start(wo_sb, wo_d[:, :].rearrange("(o p) f -> p o f", p=P))

        def layer_norm_tile(x_t, xn_t):
            """token-major [128, D] fp32 -> normalized bf16 (no gain/bias)."""
            nc.vector.memset(xn_t[0:1, 0:4], 0.0)   # claim slot: absorb WAR deps
            st = stats.tile([P, 2, 6], f32, tag="st", name=uniq("st"))
            xr = x_t.rearrange("p (s d) -> p s d", s=2)
            nc.vector.bn_stats(st[:, 0], xr[:, 0])
            nc.vector.bn_stats(st[:, 1], xr[:, 1])
            mv = stats.tile([P, 2], f32, tag="mv", name=uniq("mv"))
            nc.vector.bn_aggr(mv, st)
            rstd = stats.tile([P, 1], f32, tag="rstd", name=uniq("rstd"))
            nc.scalar.activation(rstd, mv[:, 1:2], func=AF.Sqrt, bias=eps_t,
                                 scale=1.0)
            nc.vector.reciprocal(rstd, rstd)
            nc.vector.tensor_scalar(xn_t, x_t, scalar1=mv[:, 0:1], scalar2=rstd,
                                    op0=OP.subtract, op1=OP.mult)

        def pe_transpose_to(dst_of_dj, src_t):
            """[128 tok, D] bf16 -> feature-major dst[:, dj, tok-slice]."""
            for dj in range(DC):
                pst = psum.tile([P, P], bf16, tag="ps", name=uniq("pst"))
                nc.tensor.transpose(pst, src_t[:, dj * P:(dj + 1) * P], ident)
                nc.scalar.activation(dst_of_dj(dj), pst, func=AF.Copy)

        # ---- phase 1: LN1 + transpose to feature-major ----
        # left-stack pools in reverse-death order (LIFO); x slab on the right
        lnxn = tc.alloc_tile_pool(name="lnxn", bufs=2)     # dies after LN2
        aop = tc.alloc_tile_pool(name="ao", bufs=1)
        denp = tc.alloc_tile_pool(name="den", bufs=1)
        x2p = tc.alloc_tile_pool(name="x2", bufs=2)
        xnTp = tc.alloc_tile_pool(name="xnT", bufs=1)
        vap = tc.alloc_tile_pool(name="vaug", bufs=1)
        qkp = tc.alloc_tile_pool(name="qk", bufs=4)
        expp = tc.alloc_tile_pool(name="expp", bufs=20)
        lnx = tc.alloc_tile_pool(name="lnx", bufs=1, side="right")

        xnT = xnTp.tile([P, DC, N], bf16, tag="xnT")
        xfull = lnx.tile([P, TT, D], bf16, tag="x")
        for xq in range(4):
            nc.sync.dma_start(
                xfull[:, 2 * xq:2 * xq + 2, :],
                x_d[xq * 256:(xq + 1) * 256, :].rearrange(
                    "(o p) f -> p o f", p=P))
        # V projection for tile ti needs only tile ti's transposes, so it is
        # fused one iteration behind LN1: the PE runs V matmuls while the DVE
        # does the next tile's LayerNorm (PE-dense from kernel start).
        v_aug = vap.tile([P, TT, H, DH + 1], bf16, tag="vaug")
        nc.vector.memset(v_aug[:, :, :, DH:DH + 1], 1.0)

        def emit_v(ti):
            for fh in range(2):
                psv = ps_tile()
                for dc in range(DC):
                    nc.tensor.matmul(psv, lhsT=xnT[:, dc, ti * P:(ti + 1) * P],
                                     rhs=wv_sb[:, dc, fh * 512:(fh + 1) * 512],
                                     start=(dc == 0), stop=(dc == DC - 1))
                nc.vector.tensor_tensor(
                    v_aug[:, ti, fh * 8:(fh + 1) * 8, 0:DH],
                    psv[:].rearrange("p (h d) -> p h d", d=DH),
                    bv_b[:, fh * 512:(fh + 1) * 512].rearrange(
                        "p (h d) -> p h d", d=DH),
                    OP.add)

        for ti in range(TT):
            xn_t = lnxn.tile([P, D], bf16, tag="xn", name=uniq("xn"))
            layer_norm_tile(xfull[:, ti, :], xn_t)
            pe_transpose_to(
                lambda dj, ti=ti: xnT[:, dj, ti * P:(ti + 1) * P], xn_t)
            if ti > 0:
                emit_v(ti - 1)
        emit_v(TT - 1)
        lnx.release()

        # ---- phase 3: attention, Q/K streamed per head pair ----
        aoT = aop.tile([P, DC, N], bf16, tag="aoT")

        def emit_qk(hp):
            qt_t = qkp.tile([P, N], bf16, tag="qT", name=uniq("qT"))
            kt_t = qkp.tile([P, N], bf16, tag="kT", name=uniq("kT"))
            nc.vector.memset(qt_t[0:1, 0:4], 0.0)
            nc.vector.memset(kt_t[0:1, 0:4], 0.0)
            for qh in range(2):
                psq = ps_tile()
                for dc in range(DC):
                    nc.tensor.matmul(psq, lhsT=wq_sb[:, dc, hp * P:(hp + 1) * P],
                                     rhs=xnT[:, dc, qh * 512:(qh + 1) * 512],
                                     start=(dc == 0), stop=(dc == DC - 1))
                nc.vector.tensor_scalar_add(qt_t[:, qh * 512:(qh + 1) * 512],
                                            psq, bq_sb[:, hp:hp + 1])
                psk = ps_tile()
                for dc in range(DC):
                    nc.tensor.matmul(psk, lhsT=wk_sb[:, dc, hp * P:(hp + 1) * P],
                                     rhs=xnT[:, dc, qh * 512:(qh + 1) * 512],
                                     start=(dc == 0), stop=(dc == DC - 1))
                nc.vector.tensor_scalar_add(kt_t[:, qh * 512:(qh + 1) * 512],
                                            psk, bk_sb[:, hp:hp + 1])
            return qt_t, kt_t

        for hp in range(DC):      # head pair: heads 2hp (rows 0:64), 2hp+1
            usA = [ps_tile() for _ in range(2)]
            usB = [ps_tile() for _ in range(2)]
            qt_t, kt_t = emit_qk(hp)

            # staged pipeline: the PE queue is in-order, so attn@V matmuls
            # are enqueued ~16 instructions behind the scores whose exps they
            # consume. Per query-half: scores(qh) run dense while ACT exps
            # trail; attn@V(qh) interleaves with scores(qh+1).
            ext_of = {}

            def emit_score(mc, qh, hh, r0):
                pss = ps_tile()
                nc.tensor.matmul(
                    pss,
                    lhsT=kt_t[r0:r0 + 64, mc * P:(mc + 1) * P],
                    rhs=qt_t[r0:r0 + 64, qh * 512:(qh + 1) * 512],
                    start=True, stop=True)
                ext = expp.tile([P, 512], bf16, tag="expT", name=uniq("expT"))
                nc.scalar.activation(ext, pss, func=AF.Exp)
                ext_of[(mc, qh, hh)] = ext

            def emit_av(mc, qh, hh, us):
                nc.tensor.matmul(
                    us[qh][0:DH + 1, :],
                    lhsT=v_aug[:, mc, 2 * hp + hh, :],
                    rhs=ext_of.pop((mc, qh, hh)),
                    start=(mc == 0), stop=(mc == TT - 1))

            for mc in range(TT):            # phase a: scores qh=0
                for hh, r0 in ((0, 0), (1, 64)):
                    emit_score(mc, 0, hh, r0)
            for mc in range(TT):            # phase b: attn@V qh=0 ; scores qh=1
                emit_av(mc, 0, 0, usA)
                emit_av(mc, 0, 1, usB)
                for hh, r0 in ((0, 0), (1, 64)):
                    emit_score(mc, 1, hh, r0)
            for mc in range(TT):            # phase c: attn@V qh=1
                emit_av(mc, 1, 0, usA)
                emit_av(mc, 1, 1, usB)
            # broadcast the RAW denominator row (one 1-partition copy gates
            # the PE, not two), then reciprocal runs wide (64 partitions) on
            # the broadcast result.
            for hh, r0, us in ((0, 0, usA), (1, 64, usB)):
                den = denp.tile([1, N], mybir.dt.float32r, tag="den",
                                name=uniq("den"))
                for qh in range(2):
                    with nc.allow_low_precision(
                            reason="fp32r rounding of the softmax denominator "
                                   "(~2^-13 relative) for the 1-cycle/row "
                                   "PE broadcast"):
                        nc.vector.tensor_copy(
                            den[:, qh * 512:(qh + 1) * 512],
                            us[qh][DH:DH + 1, :])
                for qh in range(2):
                    psb = ps_tile()
                    nc.tensor.matmul(
                        psb[0:64, :],
                        lhsT=ones64[:],
                        rhs=den[:, qh * 512:(qh + 1) * 512],
                        start=True, stop=True)
                    denBs = denp.tile([64, 512], f32, tag="denBs",
                                      name=uniq("denBs"))
                    nc.vector.reciprocal(denBs, psb[0:64, :])
                    nc.vector.tensor_mul(
                        aoT[r0:r0 + 64, hp, qh * 512:(qh + 1) * 512],
                        us[qh][0:64, :], denBs)

        expp.release()
        qkp.release()
        vap.release()
        xnTp.release()

        # q/k/v slots free -> start w1 loads (queue-aligned to old tenants)
        W1_PADS = globals().get("_W1_PADS", [0, 0, 0])
        w1_t = []
        for g in range(3):   # quarters 0-2 reuse the q/k/v slots right away
            qpad(W1_PADS[g])
            t = wtile([P, 2, MLP])
            nc.sync.dma_start(
                t, w1_d[g * 256:(g + 1) * 256, :].rearrange(
                    "(o p) f -> p o f", p=P))
            w1_t.append(t)

        # ---- phase 4: out-projection + LN2 + transpose ----
        hTp = tc.alloc_tile_pool(name="hT", bufs=1, side="right")
        xn2Tp = tc.alloc_tile_pool(name="xn2T", bufs=1, side="right")
        hT = hTp.tile([P, MT, N], bf16, tag="hT")
        xn2T = xn2Tp.tile([P, DC, N], bf16, tag="xn2T")

        pend_tr = [None]
        for ti in range(TT):
            x2_t = x2p.tile([P, D], bf16, tag="x2", name=uniq("x2"))
            nc.vector.memset(x2_t[0:1, 0:4], 0.0)
            for fh in range(2):
                pso = ps_tile()
                for dc in range(DC):
                    nc.tensor.matmul(pso, lhsT=aoT[:, dc, ti * P:(ti + 1) * P],
                                     rhs=wo_sb[:, dc, fh * 512:(fh + 1) * 512],
                                     start=(dc == 0), stop=(dc == DC - 1))
                nc.vector.tensor_add(x2_t[:, fh * 512:(fh + 1) * 512], pso,
                                     bo_b[:, fh * 512:(fh + 1) * 512])
            # defer this tile's transposes behind the next tile's out-proj so
            # the in-order PE queue never waits on the DVE LayerNorm chain
            if pend_tr[0] is not None:
                pend_tr[0]()
            xn2_t = lnxn.tile([P, D], bf16, tag="xn", name=uniq("xn"))
            layer_norm_tile(x2_t, xn2_t)
            pend_tr[0] = (lambda ti=ti, xt=xn2_t: pe_transpose_to(
                lambda dj: xn2T[:, dj, ti * P:(ti + 1) * P], xt))
        pend_tr[0]()

        # wout done -> load the last w1 quarter into its slot
        qpad(globals().get("_W1D_PAD", 0))
        w1d = wtile([P, 2, MLP])
        nc.sync.dma_start(
            w1d, w1_d[768:1024, :].rearrange("(o p) f -> p o f", p=P))
        w1_t.append(w1d)

        x2p.release()
        denp.release()
        aop.release()
        lnxn.release()
        w2ep = tc.alloc_tile_pool(name="w2e", bufs=2)
        w2_t = []
        for g in range(2):
            t = w2ep.tile([P, TT, D], bf16, tag="w2e", name=uniq("w2e"))
            nc.sync.dma_start(
                t, w2_d[g * 1024:(g + 1) * 1024, :].rearrange(
                    "(o p) f -> p o f", p=P))
            w2_t.append(t)

        # ---- phase 5: FFN1 (feature-major h, fused bias+gelu) ----
        for m in range(MT):
            for qh in range(2):
                psh = ps_tile()
                for dc in range(DC):
                    nc.tensor.matmul(
                        psh,
                        lhsT=w1_t[dc // 2][:, dc % 2, m * P:(m + 1) * P],
                        rhs=xn2T[:, dc, qh * 512:(qh + 1) * 512],
                        start=(dc == 0), stop=(dc == DC - 1))
                nc.scalar.activation(hT[:, m, qh * 512:(qh + 1) * 512], psh,
                                     func=gelu_func, bias=bh_sb[:, m:m + 1],
                                     scale=1.0)

        # scrub w1 slots, load w2 quarters
        for g in range(2, 4):
            t = wtile([P, TT, D])
            nc.gpsimd.dma_start(
                t, w2_d[g * 1024:(g + 1) * 1024, :].rearrange(
                    "(o p) f -> p o f", p=P))
            w2_t.append(t)

        xn2Tp.release()

        # ---- phase 6: FFN2 (token-major out) ----
        for th in range(2):
            pss = [ps_tile() for _ in range(8)]   # 4 t-tiles x 2 f-halves
            for c in range(MT):
                for tloc in range(4):
                    ti = th * 4 + tloc
                    for fh in range(2):
                        nc.tensor.matmul(
                            pss[tloc * 2 + fh],
                            lhsT=hT[:, c, ti * P:(ti + 1) * P],
                            rhs=w2_t[c // 8][:, c % 8, fh * 512:(fh + 1) * 512],
                            start=(c == 0), stop=(c == MT - 1))
            for tloc in range(4):
                ti = th * 4 + tloc
                o_t = outp.tile([P, D], f32, tag="o", name=uniq("o"))
                nc.vector.memset(o_t[0:1, 0:4], 0.0)
                for fh in range(2):
                    nc.vector.tensor_add(o_t[:, fh * 512:(fh + 1) * 512],
                                         pss[tloc * 2 + fh],
                                         b2_b[:, fh * 512:(fh + 1) * 512])
                nc.sync.dma_start(out_d[ti * P:(ti + 1) * P, :], o_t)

        w2ep.release()
        hTp.release()
        outp.release()
        wslot.release()
        psum.release()
        stats.release()
        const.release()

    nc.finalize()   # bacc legalization: wait splitting, table/library loads
    return nc


def prep_inputs(inputs):
    """Host-side weight folding + bf16 casts. Returns (shared_map, per_core_x)."""
    f = lambda k: np.asarray(inputs[k], dtype=np.float32)
    x = f("x")
    g1, b1 = f("ln1_g"), f("ln1_b")
    w_qkv, w_out, b_out = f("w_qkv"), f("w_out"), f("b_out")
    g2, b2l = f("ln2_g"), f("ln2_b")
    w1, bias1, w2, bias2 = f("w1"), f("b1"), f("w2"), f("b2")

    scale = DH ** -0.5
    wqkv_g = g1[:, None] * w_qkv
    bias_qkv = b1 @ w_qkv
    bf = ml_dtypes.bfloat16
    shared = {
        "wq": np.ascontiguousarray(wqkv_g[:, :D] * scale).astype(bf),
        "wk": np.ascontiguousarray(wqkv_g[:, D:2 * D]).astype(bf),
        "wv": np.ascontiguousarray(wqkv_g[:, 2 * D:]).astype(bf),
        "wo": w_out.astype(bf),
        "w1": (g2[:, None] * w1).astype(bf),
        "w2": w2.astype(bf),
        "bq": np.ascontiguousarray(bias_qkv[:D] * scale),
        "bk": np.ascontiguousarray(bias_qkv[D:2 * D]),
        "bv": np.ascontiguousarray(bias_qkv[2 * D:]),
        "bo": b_out.copy(),
        "bh": b2l @ w1 + bias1,
        "b2": bias2.copy(),
    }
    xs = [np.ascontiguousarray(x[i]).astype(bf) for i in range(B)]
    return shared, xs


_CACHED_NC = None


def _get_nc():
    global _CACHED_NC
    if _CACHED_NC is None:
        _CACHED_NC = build_bass()
    return _CACHED_NC


def run(inputs, trace=False):
    from concourse.bass_utils import run_bass_kernel_spmd
    nc = _get_nc()
    shared, xs = prep_inputs(inputs)
    in_maps = [{**shared, "x": xs[i]} for i in range(NCORES)]
    res = run_bass_kernel_spmd(nc, in_maps, list(range(NCORES)), trace=trace)
    out = np.stack([np.asarray(res.results[i]["out"]) for i in range(NCORES)], 0)
    return out.astype(np.float32), res


def kernel(**inputs):
    out, _ = run(inputs)
    return out



# revision 5
# speedup vs baseline: 1.0806x; 1.0806x over previous
"""Trainium2 Bass kernel for a dense transformer block (LN -> QKV -> attention ->
out-proj -> LN -> FFN with exact GELU, no residuals).

Sharding: pure data parallelism — batch 8 across 8 NeuronCores, one batch element
per core. Each core runs the full block on its [1024, 1024] token slab.

On-chip dataflow (per core):
  - LN1 in token-major fp32 (bn_stats); normalized output cast to bf16 and
    transposed to feature-major xnT [D, tokens] on the PE (bf16 transpose-mode).
  - All GEMMs in bf16 with fp32 PSUM accumulation. LayerNorm gains fold into the
    weights on the host; LN biases fold into per-feature GEMM biases. The
    1/sqrt(dh) attention scale folds into the Q projection.
  - Q/K produced feature-major per head pair (streamed), V token-major with a
    ones-column per head so attention@V also yields the softmax denominator.
  - Scores are computed k-major (scoresT) so the exp output feeds attention@V
    directly with no transpose; softmax skips max-subtraction (|scores| < ~3).
  - Denominator reciprocal is partition-broadcast by GPSIMD (no DMA).
  - x2 token-major, LN2 like LN1, FFN1 feature-major with fused bias+GELU on
    ACT, FFN2 token-major.

DMA discipline (hardware caps each DMA descriptor at ~2 sem waits, and Tile
adds one flow-control wait per DMA once queues saturate, so every DMA must
carry at most ONE data wait):
  - Weights live in 4 permanent 16KB "wslot" tag slots (wq/wk/wv/wout ->
    w1 quarters -> w2 quarters). Between tenants the slot is fully memset by
    GPSIMD: engine instructions have no wait caps and become the slot's sole
    last-writer, so the next load carries a single WAW wait.
  - Activation pools recycle SBUF freely but are only ever written by compute
    engines; output staging is a permanent pool so stores see only DVE deps.
"""

import numpy as np
import ml_dtypes

B, N, D = 8, 1024, 1024
H, DH = 16, 64
MLP = 4096
EPS = 1e-5
P = 128
NCORES = 8
TT = N // P    # 8 token tiles
DC = D // P    # 8 d-chunks
MT = MLP // P  # 32 mlp tiles


def build_bass(gelu_mode="gelu"):
    import concourse.bass as bass
    import concourse.mybir as mybir
    import concourse.tile as tile
    from concourse import bacc
    from concourse.masks import make_identity

    f32 = mybir.dt.float32
    bf16 = mybir.dt.bfloat16
    AF = mybir.ActivationFunctionType
    OP = mybir.AluOpType

    nc = bacc.Bacc()

    x_d = nc.declare_dram_parameter("x", [N, D], bf16, isOutput=False)
    wq_d = nc.declare_dram_parameter("wq", [D, D], bf16, isOutput=False)
    wk_d = nc.declare_dram_parameter("wk", [D, D], bf16, isOutput=False)
    wv_d = nc.declare_dram_parameter("wv", [D, D], bf16, isOutput=False)
    wo_d = nc.declare_dram_parameter("wo", [D, D], bf16, isOutput=False)
    w1_d = nc.declare_dram_parameter("w1", [D, MLP], bf16, isOutput=False)
    w2_d = nc.declare_dram_parameter("w2", [MLP, D], bf16, isOutput=False)
    bq_d = nc.declare_dram_parameter("bq", [D], f32, isOutput=False)
    bk_d = nc.declare_dram_parameter("bk", [D], f32, isOutput=False)
    bv_d = nc.declare_dram_parameter("bv", [D], f32, isOutput=False)
    bo_d = nc.declare_dram_parameter("bo", [D], f32, isOutput=False)
    bh_d = nc.declare_dram_parameter("bh", [MLP], f32, isOutput=False)
    b2_d = nc.declare_dram_parameter("b2", [D], f32, isOutput=False)
    out_d = nc.declare_dram_parameter("out", [N, D], f32, isOutput=True)

    gelu_func = AF.Gelu if gelu_mode == "gelu" else AF.Identity

    with tile.TileContext(nc) as tc:
        # ---- permanent pools (left stack bottom) ----
        const = tc.alloc_tile_pool(name="const", bufs=1)
        stats = tc.alloc_tile_pool(name="stats", bufs=4)
        psum = tc.alloc_tile_pool(name="psum", bufs=8, space="PSUM")
        wslot = tc.alloc_tile_pool(name="wslot", bufs=4)   # 4 x 16KB weight slots
        outp = tc.alloc_tile_pool(name="outp", bufs=2)

        counter = [0]

        def uniq(prefix):
            counter[0] += 1
            return f"{prefix}{counter[0]}"

        def ps_tile():
            return psum.tile([P, 512], f32, tag="ps", name=uniq("ps"))

        def wtile(shape):
            return wslot.tile(shape, bf16, tag="w", name=uniq("w"))

        def qpad(n, after_ap=None):
            """Emit n dummy HWDGE DMAs (virgin tiny targets) to rotate the
            round-robin queue assignment, so the next real load lands on the
            same queue as its slot's previous tenant load (merging the
            slot-WAW wait with the prior-same-queue wait). after_ap pins the
            pads' schedule position via a RAW dependency."""
            for _ in range(n):
                if after_ap is None:
                    d = const.tile([1, 4], f32, tag=uniq("pad"),
                                   name=uniq("padt"))
                    nc.sync.dma_start(d, bq_d[None, 0:4])
                else:
                    d = const.tile([1, 4], bf16, tag=uniq("pad"),
                                   name=uniq("padt"))
                    nc.sync.dma_start(d, after_ap)

        eps_t = const.tile([P, 1], f32, tag="eps")
        nc.vector.memset(eps_t, EPS)
        bq_sb = const.tile([P, DC], f32, tag="bq")
        nc.sync.dma_start(bq_sb, bq_d[:].rearrange("(o p) -> p o", p=P))
        bk_sb = const.tile([P, DC], f32, tag="bk")
        nc.sync.dma_start(bk_sb, bk_d[:].rearrange("(o p) -> p o", p=P))
        bh_sb = const.tile([P, MT], f32, tag="bh")
        nc.sync.dma_start(bh_sb, bh_d[:].rearrange("(o p) -> p o", p=P))
        bv_b = const.tile([P, D], bf16, tag="bvb")
        nc.gpsimd.dma_start(bv_b, bv_d[None, :].to_broadcast([P, D]))
        bo_b = const.tile([P, D], bf16, tag="bob")
        nc.gpsimd.dma_start(bo_b, bo_d[None, :].to_broadcast([P, D]))
        b2_b = const.tile([P, D], bf16, tag="b2b")
        nc.gpsimd.dma_start(b2_b, b2_d[None, :].to_broadcast([P, D]))
        ident = const.tile([P, P], bf16, tag="ident")
        make_identity(nc, ident)
        ones64f = const.tile([1, 64], f32, tag="ones64f")
        nc.vector.memset(ones64f, 1.0)
        ones64 = const.tile([1, 64], mybir.dt.float32r, tag="ones64")
        nc.vector.tensor_copy(ones64, ones64f)

        def layer_norm_tile(x_t, xn_t):
            """token-major [128, D] fp32 -> normalized bf16 (no gain/bias)."""
            nc.vector.memset(xn_t[0:1, 0:4], 0.0)   # claim slot: absorb WAR deps
            st = stats.tile([P, 2, 6], f32, tag="st", name=uniq("st"))
            xr = x_t.rearrange("p (s d) -> p s d", s=2)
            nc.vector.bn_stats(st[:, 0], xr[:, 0])
            nc.vector.bn_stats(st[:, 1], xr[:, 1])
            mv = stats.tile([P, 2], f32, tag="mv", name=uniq("mv"))
            nc.vector.bn_aggr(mv, st)
            rstd = stats.tile([P, 1], f32, tag="rstd", name=uniq("rstd"))
            nc.scalar.activation(rstd, mv[:, 1:2], func=AF.Sqrt, bias=eps_t,
                                 scale=1.0)
            nc.vector.reciprocal(rstd, rstd)
            nc.vector.tensor_scalar(xn_t, x_t, scalar1=mv[:, 0:1], scalar2=rstd,
                                    op0=OP.subtract, op1=OP.mult)

        def pe_transpose_to(dst_of_dj, src_t):
            """[128 tok, D] bf16 -> feature-major dst[:, dj, tok-slice]."""
            for dj in range(DC):
                pst = psum.tile([P, P], bf16, tag="ps", name=uniq("pst"))
                nc.tensor.transpose(pst, src_t[:, dj * P:(dj + 1) * P], ident)
                nc.scalar.activation(dst_of_dj(dj), pst, func=AF.Copy)

        # ---- phase 1: LN1 + transpose to feature-major ----
        # left-stack pools in reverse-death order (LIFO); x slab on the right
        lnxn = tc.alloc_tile_pool(name="lnxn", bufs=2)     # dies after LN2
        aop = tc.alloc_tile_pool(name="ao", bufs=1)
        denp = tc.alloc_tile_pool(name="den", bufs=1)
        x2p = tc.alloc_tile_pool(name="x2", bufs=2)
        xnTp = tc.alloc_tile_pool(name="xnT", bufs=1)
        vap = tc.alloc_tile_pool(name="vaug", bufs=1)
        qkp = tc.alloc_tile_pool(name="qk", bufs=4)
        expp = tc.alloc_tile_pool(name="expp", bufs=20)
        lnx = tc.alloc_tile_pool(name="lnx", bufs=1, side="right")

        xnT = xnTp.tile([P, DC, N], bf16, tag="xnT")
        xfull = lnx.tile([P, TT, D], bf16, tag="x")
        # x tiles FIRST (LN1 blocks on tile 0), then weights in first-use
        # order (wv for the fused V projection, wq/wk at attention, wo last)
        for xq in range(TT):
            nc.sync.dma_start(
                xfull[:, xq:xq + 1, :],
                x_d[xq * P:(xq + 1) * P, :].rearrange(
                    "(o p) f -> p o f", p=P))
        wv_sb = wtile([P, DC, D])
        nc.sync.dma_start(wv_sb, wv_d[:, :].rearrange("(o p) f -> p o f", p=P))
        wq_sb = wtile([P, DC, D])
        nc.sync.dma_start(wq_sb, wq_d[:, :].rearrange("(o p) f -> p o f", p=P))
        wk_sb = wtile([P, DC, D])
        nc.sync.dma_start(wk_sb, wk_d[:, :].rearrange("(o p) f -> p o f", p=P))
        wo_sb = wtile([P, DC, D])
        nc.sync.dma_start(wo_sb, wo_d[:, :].rearrange("(o p) f -> p o f", p=P))
        # V projection for tile ti needs only tile ti's transposes, so it is
        # fused one iteration behind LN1: the PE runs V matmuls while the DVE
        # does the next tile's LayerNorm (PE-dense from kernel start).
        v_aug = vap.tile([P, TT, H, DH + 1], bf16, tag="vaug")
        nc.vector.memset(v_aug[:, :, :, DH:DH + 1], 1.0)

        def emit_v(ti):
            for fh in range(2):
                psv = ps_tile()
                for dc in range(DC):
                    nc.tensor.matmul(psv, lhsT=xnT[:, dc, ti * P:(ti + 1) * P],
                                     rhs=wv_sb[:, dc, fh * 512:(fh + 1) * 512],
                                     start=(dc == 0), stop=(dc == DC - 1))
                nc.vector.tensor_tensor(
                    v_aug[:, ti, fh * 8:(fh + 1) * 8, 0:DH],
                    psv[:].rearrange("p (h d) -> p h d", d=DH),
                    bv_b[:, fh * 512:(fh + 1) * 512].rearrange(
                        "p (h d) -> p h d", d=DH),
                    OP.add)

        for ti in range(TT):
            xn_t = lnxn.tile([P, D], bf16, tag="xn", name=uniq("xn"))
            layer_norm_tile(xfull[:, ti, :], xn_t)
            pe_transpose_to(
                lambda dj, ti=ti: xnT[:, dj, ti * P:(ti + 1) * P], xn_t)
            if ti > 0:
                emit_v(ti - 1)
        emit_v(TT - 1)
        lnx.release()

        # ---- phase 3: attention, Q/K streamed per head pair ----
        aoT = aop.tile([P, DC, N], bf16, tag="aoT")

        def emit_qk(hp):
            qt_t = qkp.tile([P, N], bf16, tag="qT", name=uniq("qT"))
            kt_t = qkp.tile([P, N], bf16, tag="kT", name=uniq("kT"))
            nc.vector.memset(qt_t[0:1, 0:4], 0.0)
            nc.vector.memset(kt_t[0:1, 0:4], 0.0)
            for qh in range(2):
                psq = ps_tile()
                for dc in range(DC):
                    nc.tensor.matmul(psq, lhsT=wq_sb[:, dc, hp * P:(hp + 1) * P],
                                     rhs=xnT[:, dc, qh * 512:(qh + 1) * 512],
                                     start=(dc == 0), stop=(dc == DC - 1))
                nc.vector.tensor_scalar_add(qt_t[:, qh * 512:(qh + 1) * 512],
                                            psq, bq_sb[:, hp:hp + 1])
                psk = ps_tile()
                for dc in range(DC):
                    nc.tensor.matmul(psk, lhsT=wk_sb[:, dc, hp * P:(hp + 1) * P],
                                     rhs=xnT[:, dc, qh * 512:(qh + 1) * 512],
                                     start=(dc == 0), stop=(dc == DC - 1))
                nc.vector.tensor_scalar_add(kt_t[:, qh * 512:(qh + 1) * 512],
                                            psk, bk_sb[:, hp:hp + 1])
            return qt_t, kt_t

        # Per head-pair schedule (all engines pipelined, no PE gaps):
        #   PE : [phase a sc qh0][phase b av qh0 + sc qh1][phase c av qh1]
        #        [bcast qh0][QK proj hp+1][bcast qh1][phase a' ...]
        #   ACT: exps trail scores by a few tiles.
        #   DVE: den copies -> recip+mul qh0 -> den copies qh1 -> qk bias
        #        adds -> recip+mul qh1.  The reciprocal runs on the PE-
        #        broadcast [64,512] tile, off the PE critical path (the
        #        next hp's QK matmuls are already enqueued ahead of it).
        qk_cur = emit_qk(0)

        def den_copy(us, hh, qh, den):
            with nc.allow_low_precision(
                    reason="fp32r rounding of the softmax denominator "
                           "(~2^-13 relative) for the 1-cycle/row "
                           "PE broadcast"):
                nc.vector.tensor_copy(
                    den[:, hh * 512:(hh + 1) * 512],
                    us[qh][DH:DH + 1, :])

        def den_finish(us_of_hh, hp, qh, den):
            """broadcast raw den -> reciprocal wide -> scale into aoT."""
            for hh, r0 in ((0, 0), (1, 64)):
                psb = ps_tile()
                nc.tensor.matmul(
                    psb[0:64, :],
                    lhsT=ones64[:],
                    rhs=den[:, hh * 512:(hh + 1) * 512],
                    start=True, stop=True)
                denBs = denp.tile([64, 512], f32, tag="denBs",
                                  name=uniq("denBs"))
                nc.vector.reciprocal(denBs, psb[0:64, :])
                nc.vector.tensor_mul(
                    aoT[r0:r0 + 64, hp, qh * 512:(qh + 1) * 512],
                    us_of_hh[hh][qh][0:64, :], denBs)

        for hp in range(DC):      # head pair: heads 2hp (rows 0:64), 2hp+1
            usA = [ps_tile() for _ in range(2)]
            usB = [ps_tile() for _ in range(2)]
            qt_t, kt_t = qk_cur

            ext_of = {}

            def emit_score(mc, qh, hh, r0):
                pss = ps_tile()
                nc.tensor.matmul(
                    pss,
                    lhsT=kt_t[r0:r0 + 64, mc * P:(mc + 1) * P],
                    rhs=qt_t[r0:r0 + 64, qh * 512:(qh + 1) * 512],
                    start=True, stop=True)
                ext = expp.tile([P, 512], bf16, tag="expT", name=uniq("expT"))
                nc.scalar.activation(ext, pss, func=AF.Exp)
                ext_of[(mc, qh, hh)] = ext

            def emit_av(mc, qh, hh, us):
                nc.tensor.matmul(
                    us[qh][0:DH + 1, :],
                    lhsT=v_aug[:, mc, 2 * hp + hh, :],
                    rhs=ext_of.pop((mc, qh, hh)),
                    start=(mc == 0), stop=(mc == TT - 1))

            for mc in range(TT):            # phase a: scores qh=0
                for hh, r0 in ((0, 0), (1, 64)):
                    emit_score(mc, 0, hh, r0)
            for mc in range(TT):            # phase b: attn@V qh=0 ; scores qh=1
                emit_av(mc, 0, 0, usA)
                emit_av(mc, 0, 1, usB)
                for hh, r0 in ((0, 0), (1, 64)):
                    emit_score(mc, 1, hh, r0)
            # qh=0 denominator rows are final after phase b; copy them now so
            # the qh=0 broadcasts can issue right behind phase c.
            den0 = denp.tile([1, N], mybir.dt.float32r, tag="den",
                             name=uniq("den"))
            den_copy(usA, 0, 0, den0)
            den_copy(usB, 1, 0, den0)
            for mc in range(TT):            # phase c: attn@V qh=1
                emit_av(mc, 1, 0, usA)
                emit_av(mc, 1, 1, usB)
            den_finish({0: usA, 1: usB}, hp, 0, den0)
            den1 = denp.tile([1, N], mybir.dt.float32r, tag="den",
                             name=uniq("den"))
            den_copy(usA, 0, 1, den1)
            den_copy(usB, 1, 1, den1)
            # next head pair's Q/K projections keep the PE dense while the
            # DVE works through the qh=0 reciprocal chain
            if hp + 1 < DC:
                qk_cur = emit_qk(hp + 1)
            den_finish({0: usA, 1: usB}, hp, 1, den1)

        expp.release()
        qkp.release()
        vap.release()
        xnTp.release()

        # q/k/v slots free -> start w1 loads (queue-aligned to old tenants)
        W1_PADS = globals().get("_W1_PADS", [0, 0, 0])
        w1_t = []
        for g in range(3):   # quarters 0-2 reuse the q/k/v slots right away
            qpad(W1_PADS[g])
            t = wtile([P, 2, MLP])
            nc.sync.dma_start(
                t, w1_d[g * 256:(g + 1) * 256, :].rearrange(
                    "(o p) f -> p o f", p=P))
            w1_t.append(t)

        # ---- phase 4: out-projection + LN2 + transpose ----
        hTp = tc.alloc_tile_pool(name="hT", bufs=1, side="right")
        xn2Tp = tc.alloc_tile_pool(name="xn2T", bufs=1, side="right")
        hT = hTp.tile([P, MT, N], bf16, tag="hT")
        xn2T = xn2Tp.tile([P, DC, N], bf16, tag="xn2T")

        pend_tr = [None]
        for ti in range(TT):
            x2_t = x2p.tile([P, D], bf16, tag="x2", name=uniq("x2"))
            nc.vector.memset(x2_t[0:1, 0:4], 0.0)
            for fh in range(2):
                pso = ps_tile()
                for dc in range(DC):
                    nc.tensor.matmul(pso, lhsT=aoT[:, dc, ti * P:(ti + 1) * P],
                                     rhs=wo_sb[:, dc, fh * 512:(fh + 1) * 512],
                                     start=(dc == 0), stop=(dc == DC - 1))
                nc.vector.tensor_add(x2_t[:, fh * 512:(fh + 1) * 512], pso,
                                     bo_b[:, fh * 512:(fh + 1) * 512])
            # defer this tile's transposes behind the next tile's out-proj so
            # the in-order PE queue never waits on the DVE LayerNorm chain
            if pend_tr[0] is not None:
                pend_tr[0]()
            xn2_t = lnxn.tile([P, D], bf16, tag="xn", name=uniq("xn"))
            layer_norm_tile(x2_t, xn2_t)
            pend_tr[0] = (lambda ti=ti, xt=xn2_t: pe_transpose_to(
                lambda dj: xn2T[:, dj, ti * P:(ti + 1) * P], xt))
        pend_tr[0]()

        # wout done -> load the last w1 quarter into its slot
        qpad(globals().get("_W1D_PAD", 0))
        w1d = wtile([P, 2, MLP])
        nc.sync.dma_start(
            w1d, w1_d[768:1024, :].rearrange("(o p) f -> p o f", p=P))
        w1_t.append(w1d)

        x2p.release()
        denp.release()
        aop.release()
        lnxn.release()
        w2ep = tc.alloc_tile_pool(name="w2e", bufs=2)
        w2_t = []
        for g in range(2):
            t = w2ep.tile([P, TT, D], bf16, tag="w2e", name=uniq("w2e"))
            nc.sync.dma_start(
                t, w2_d[g * 1024:(g + 1) * 1024, :].rearrange(
                    "(o p) f -> p o f", p=P))
            w2_t.append(t)

        # ---- phase 5: FFN1 (feature-major h, fused bias+gelu) ----
        for m in range(MT):
            for qh in range(2):
                psh = ps_tile()
                for dc in range(DC):
                    nc.tensor.matmul(
                        psh,
                        lhsT=w1_t[dc // 2][:, dc % 2, m * P:(m + 1) * P],
                        rhs=xn2T[:, dc, qh * 512:(qh + 1) * 512],
                        start=(dc == 0), stop=(dc == DC - 1))
                nc.scalar.activation(hT[:, m, qh * 512:(qh + 1) * 512], psh,
                                     func=gelu_func, bias=bh_sb[:, m:m + 1],
                                     scale=1.0)

        # scrub w1 slots, load w2 quarters
        for g in range(2, 4):
            t = wtile([P, TT, D])
            nc.gpsimd.dma_start(
                t, w2_d[g * 1024:(g + 1) * 1024, :].rearrange(
                    "(o p) f -> p o f", p=P))
            w2_t.append(t)

        xn2Tp.release()

        # ---- phase 6: FFN2 (token-major out) ----
        # two t-tiles in flight (4 PSUM banks); bias-add + store of tile i
        # overlaps the 32-chunk accumulation of tile i+1, so only the last
        # tile's store is exposed at the end.
        for tg in range(4):
            pss = [ps_tile() for _ in range(4)]   # 2 t-tiles x 2 f-halves
            for c in range(MT):
                for tloc in range(2):
                    ti = tg * 2 + tloc
                    for fh in range(2):
                        nc.tensor.matmul(
                            pss[tloc * 2 + fh],
                            lhsT=hT[:, c, ti * P:(ti + 1) * P],
                            rhs=w2_t[c // 8][:, c % 8, fh * 512:(fh + 1) * 512],
                            start=(c == 0), stop=(c == MT - 1))
            for tloc in range(2):
                ti = tg * 2 + tloc
                o_t = outp.tile([P, D], f32, tag="o", name=uniq("o"))
                nc.vector.memset(o_t[0:1, 0:4], 0.0)
                for fh in range(2):
                    nc.vector.tensor_add(o_t[:, fh * 512:(fh + 1) * 512],
                                         pss[tloc * 2 + fh],
                                         b2_b[:, fh * 512:(fh + 1) * 512])
                nc.sync.dma_start(out_d[ti * P:(ti + 1) * P, :], o_t)

        w2ep.release()
        hTp.release()
        outp.release()
        wslot.release()
        psum.release()
        stats.release()
        const.release()

    nc.finalize()   # bacc legalization: wait splitting, table/library loads
    return nc


def prep_inputs(inputs):
    """Host-side weight folding + bf16 casts. Returns (shared_map, per_core_x)."""
    f = lambda k: np.asarray(inputs[k], dtype=np.float32)
    x = f("x")
    g1, b1 = f("ln1_g"), f("ln1_b")
    w_qkv, w_out, b_out = f("w_qkv"), f("w_out"), f("b_out")
    g2, b2l = f("ln2_g"), f("ln2_b")
    w1, bias1, w2, bias2 = f("w1"), f("b1"), f("w2"), f("b2")

    scale = DH ** -0.5
    wqkv_g = g1[:, None] * w_qkv
    bias_qkv = b1 @ w_qkv
    bf = ml_dtypes.bfloat16
    shared = {
        "wq": np.ascontiguousarray(wqkv_g[:, :D] * scale).astype(bf),
        "wk": np.ascontiguousarray(wqkv_g[:, D:2 * D]).astype(bf),
        "wv": np.ascontiguousarray(wqkv_g[:, 2 * D:]).astype(bf),
        "wo": w_out.astype(bf),
        "w1": (g2[:, None] * w1).astype(bf),
        "w2": w2.astype(bf),
        "bq": np.ascontiguousarray(bias_qkv[:D] * scale),
        "bk": np.ascontiguousarray(bias_qkv[D:2 * D]),
        "bv": np.ascontiguousarray(bias_qkv[2 * D:]),
        "bo": b_out.copy(),
        "bh": b2l @ w1 + bias1,
        "b2": bias2.copy(),
    }
    xs = [np.ascontiguousarray(x[i]).astype(bf) for i in range(B)]
    return shared, xs


_CACHED_NC = None


def _get_nc():
    global _CACHED_NC
    if _CACHED_NC is None:
        _CACHED_NC = build_bass()
    return _CACHED_NC


def run(inputs, trace=False):
    from concourse.bass_utils import run_bass_kernel_spmd
    nc = _get_nc()
    shared, xs = prep_inputs(inputs)
    in_maps = [{**shared, "x": xs[i]} for i in range(NCORES)]
    res = run_bass_kernel_spmd(nc, in_maps, list(range(NCORES)), trace=trace)
    out = np.stack([np.asarray(res.results[i]["out"]) for i in range(NCORES)], 0)
    return out.astype(np.float32), res


def kernel(**inputs):
    out, _ = run(inputs)
    return out

